# revision 16
# baseline (speedup 1.0000x reference)
"""Butterworth bandpass (cascaded biquad IIR) Trainium2 kernel.

Problem: y = sosfilt(sos, x) over x[32, 64, 4096] fp32 -- 2048 independent
signals, 4 cascaded DF2T biquads, sequential over T=4096.

Strategy (exact block-parallel reformulation, bf16 data path):
  The cascade is a linear state-space system (A[8,8], B, C, D).  Split T into
  blocks of L=128, grouped in windows of R=4 blocks.  With s = state at the
  window entry, for block r of the window (operators precomputed on host in
  float64 from the 24 sos coefficients):
      y_r = Th @ x_r + sum_{r'<r} (Z A_L^{r-r'-1} F) @ x_{r'} + (Z A_L^r) @ s
      s'  = A_L^R @ s + sum_r (A_L^{R-1-r} F) @ x_r
  All device work is TensorE matmuls over [signal, time] tiles in bf16
  (1 cyc/row at any free size; fp32 PSUM accumulation):
    - the host pre-transposes x into xT block layout [128 time, w, r, sig],
      so the device does no transposes at all and input DMAs are flat
      contiguous 2KB-per-partition lines;
    - one fused rhs table THW[128, 512] = [Th | ZF | ZA_LF | ZA_L^2F] turns
      conv + intra-window cross-block corrections into a single accumulated
      matmul per source block (lhsT = xT_r, N = 512-128r);
    - entry-state corrections for all 4 blocks come from one matmul with
      rhs ZA[8, 512] (lhsT = s);
    - the state update accumulates in a [8, 256] psum; the per-window PE
      order is (g0: ZA+conv) (state) (g1: ZA+conv) so the cross-window
      state copy lands while g1 streams.
  y is written back in a partition-major bf16 layout (flat 1KB DMA lines)
  and un-permuted + upcast on the host.  2048 signals are sharded 256 per
  NeuronCore (two groups of 128 output partitions).
"""

import ml_dtypes
import numpy as np

import concourse.bass as bass
import concourse.tile as tile
from concourse import bacc
from concourse import mybir
from concourse.bass_utils import run_bass_kernel_spmd

FP32 = mybir.dt.float32
BF16 = mybir.dt.bfloat16
NPBF16 = ml_dtypes.bfloat16

P = 128            # partition width == time-block length
T = 4096
NCORES = 8
NSIG = 2048        # 32*64 independent signals
SPC = NSIG // NCORES   # 256 signals per core
NST = 8            # state dim of the 4-biquad cascade
R = 4              # blocks per window
W = P * R          # 512 time steps per window
NW = T // W        # 8 windows


# ----------------------------------------------------------------------------
# host-side: derive block-filter matrices from sos
# ----------------------------------------------------------------------------

def _build_system(sos):
    """Cascade of biquads (DF2T) -> single state space (A, B, C, D), float64."""
    sos = np.asarray(sos, dtype=np.float64)
    A = np.zeros((0, 0))
    B = np.zeros((0,))
    C = np.zeros((0,))
    D = 1.0
    for (b0, b1, b2, _one, a1, a2) in sos:
        As = np.array([[-a1, 1.0], [-a2, 0.0]])
        Bs = np.array([b1 - a1 * b0, b2 - a2 * b0])
        Cs = np.array([1.0, 0.0])
        Ds = b0
        n = A.shape[0]
        Anew = np.zeros((n + 2, n + 2))
        Anew[:n, :n] = A
        Anew[n:, :n] = np.outer(Bs, C)
        Anew[n:, n:] = As
        A = Anew
        B = np.concatenate([B, Bs * D])
        C = np.concatenate([Ds * C, Cs])
        D = Ds * D
    return A, B, C, D


def _balance(A, B, C):
    """Square-root balanced realization: both gramians become diagonal and
    equal, minimizing intermediate-magnitude disparity (important because
    bf16 matmul operands are rounded; unbalanced states reach |s|~650 and
    the rounding noise then dwarfs the O(1) output)."""
    P = np.outer(B, B)
    Ak = A.copy()
    for _ in range(64):
        P = P + Ak @ P @ Ak.T
        Ak = Ak @ Ak
    Q = np.outer(C, C)
    Ak = A.copy()
    for _ in range(64):
        Q = Q + Ak.T @ Q @ Ak
        Ak = Ak @ Ak
    Rc = np.linalg.cholesky(P + 1e-30 * np.eye(len(B)))
    M = Rc.T @ Q @ Rc
    lam, U = np.linalg.eigh(M)
    lam = np.maximum(lam, 1e-30)
    Tm = Rc @ U @ np.diag(lam ** -0.25)
    Ti = np.diag(lam ** 0.25) @ U.T @ np.linalg.inv(Rc)
    return Ti @ A @ Tm, Ti @ B, C @ Tm


def _build_matrices(sos):
    """Window-fused operator tables, float64 -> caller casts to bf16.

    THW[128, 512]: cols [128d:128d+128] = Th (d=0) or (Z A_L^(d-1) F)^T (d>=1)
    ZA [8, 512]:   cols [128r:128r+128] = (Z A_L^r)^T
    FTR[128, 32]:  cols [8r:8r+8]       = ((A_L^(R-1-r)) F)^T
    A4T[8, 8]:     (A_L^R)^T
    """
    A, B, C, D = _build_system(sos)
    A, B, C = _balance(A, B, C)
    ns = A.shape[0]
    assert ns == NST

    h = np.zeros(P)
    h[0] = D
    An = np.eye(ns)
    for k in range(1, P):
        h[k] = C @ An @ B
        An = An @ A
    Th = np.zeros((P, P))
    for m in range(P):
        Th[m, m:] = h[: P - m]

    Z = np.zeros((P, ns))
    CAn = C.copy()
    for n in range(P):
        Z[n] = CAn
        CAn = CAn @ A

    F = np.zeros((ns, P))
    AmB = B.copy()
    for m in range(P - 1, -1, -1):
        F[:, m] = AmB
        AmB = A @ AmB

    AL = np.linalg.matrix_power(A, P)

    THW = np.zeros((P, R * P))
    THW[:, :P] = Th
    for d in range(1, R):
        THW[:, d * P:(d + 1) * P] = (Z @ np.linalg.matrix_power(AL, d - 1) @ F).T
    ZA = np.zeros((ns, R * P))
    for r in range(R):
        ZA[:, r * P:(r + 1) * P] = (Z @ np.linalg.matrix_power(AL, r)).T
    FTR = np.zeros((P, R * NST))
    for r in range(R):
        FTR[:, r * NST:(r + 1) * NST] = (np.linalg.matrix_power(AL, R - 1 - r) @ F).T
    A4T = np.linalg.matrix_power(AL, R).T
    return THW, ZA, FTR, A4T


# ----------------------------------------------------------------------------
# device kernel
# ----------------------------------------------------------------------------

# x chunk split (windows per DMA) interleaved across the two HWDGE engines:
# sync gets w0 alone so the first conv can start ASAP.
XCHUNKS = [  # (engine_idx, [windows])
    (0, [0]),
    (1, [1]),
    (0, [2, 3]),
    (1, [4, 5]),
    (0, [6, 7]),
]


def _build_nc():
    nc = bacc.Bacc("TRN2", target_bir_lowering=False)
    # xt layout: [128 tpos, (8 w, 4 r, 256 s)]  -- element [p, w, r, s]
    xt_d = nc.dram_tensor("xt", [P, NW * R * SPC], BF16, kind="ExternalInput").ap()
    # [THW | FTR | ZA (rows 0:8) | A4T (rows 0:8)] in one table
    CT = R * P + R * NST + R * P + NST
    ctab_d = nc.dram_tensor("ctab", [P, CT], BF16, kind="ExternalInput").ap()
    # y layout: [128 ps, (8 w, 2 g, 512 c)] -- element [p, w, g, c]
    y_d = nc.dram_tensor("y", [P, NW * 2 * W], BF16, kind="ExternalOutput").ap()

    NWARM = 24  # p-state warm-up matmuls bridging the initial DMA wait

    WCOL = R * SPC  # xt columns per window (1024)

    with tile.TileContext(nc) as tc:
        with (
            tc.tile_pool(name="consts", bufs=1) as consts,
            tc.tile_pool(name="ypool", bufs=3) as ypool,
            tc.tile_pool(name="spool", bufs=3) as spool,
            tc.tile_pool(name="py", bufs=2, space="PSUM") as pyp,
            tc.tile_pool(name="ps", bufs=2, space="PSUM") as psp,
            tc.tile_pool(name="pw", bufs=1, space="PSUM") as pwp,
        ):
            dma_eng = (nc.sync, nc.scalar)
            # constant tables first (tiny; they gate the first matmuls),
            # then the x window chunks
            ctab_sb = consts.tile([P, CT], BF16)
            nc.sync.dma_start(ctab_sb, ctab_d)
            thw_sb = ctab_sb[:, 0:R * P]
            ftr_sb = ctab_sb[:, R * P:R * P + R * NST]
            za_sb = ctab_sb[0:NST, R * P + R * NST:2 * R * P + R * NST]
            a4t_sb = ctab_sb[0:NST, 2 * R * P + R * NST:]
            xw_sb = [None] * NW
            for eng, ws in XCHUNKS:
                t = consts.tile([P, len(ws) * WCOL], BF16, name=f"xw{ws[0]}")
                dma_eng[eng].dma_start(
                    t, xt_d[:, ws[0] * WCOL:(ws[-1] + 1) * WCOL]
                )
                for i, w in enumerate(ws):
                    xw_sb[w] = t[:, i * WCOL:(i + 1) * WCOL]

            # warm-up: keep the PE continuously busy through the DVFS ramp
            # (max clock needs ~3us of gap-free execution) while the first
            # x/ctab DMAs are in flight.  Zeroed operands, result unused.
            warm_sb = consts.tile([P, 3 * P], BF16, name="warm")
            nc.gpsimd.memset(warm_sb, 0)
            psum_warm = pwp.tile([P, 3 * P], FP32, tag="warm")
            for _ in range(NWARM):
                nc.tensor.matmul(
                    psum_warm, warm_sb[:, 0:P], warm_sb, start=True, stop=True,
                )

            s_prev = None  # window 0 enters with zero state

            for w in range(NW):
                xw = xw_sb[w]

                def xt_g(r, g):  # [128, 128] lhsT for group g, block r
                    return xw[:, r * SPC + g * P: r * SPC + (g + 1) * P]

                def xt_full(r):  # [128, 256] rhs for the state update
                    return xw[:, r * SPC:(r + 1) * SPC]

                psum_y = [
                    pyp.tile([P, W], FP32, tag=f"py{g}", name=f"py{g}")
                    for g in (0, 1)
                ]
                y_sb = ypool.tile([P, 2 * W], BF16, tag="y", name="y_sb")

                # y psum for group g: ZA @ s + conv, every matmul split at
                # column 256 -- N=512 matmuls stream at 1.5 cyc/row on HW
                # while N<=384 run at 1.0, so two N=256 halves are faster.
                def y_group(g):
                    first = s_prev is None
                    if not first:
                        nc.tensor.matmul(
                            psum_y[g], s_prev[:, g * P:(g + 1) * P],
                            za_sb, start=True, stop=False,
                        )
                    for r in range(R):
                        nc.tensor.matmul(
                            psum_y[g][:, r * P:],
                            xt_g(r, g),
                            thw_sb[:, : (R - r) * P],
                            start=(first and r == 0),
                            stop=(r == R - 1),
                        )

                y_group(0)

                # state update (before g1 so the s copy hides under g1);
                # a4t last so its tiny weight load hides under ftr3
                psum_s = psp.tile([NST, 2 * P], FP32, tag="ps")
                for r in range(R):
                    nc.tensor.matmul(
                        psum_s, ftr_sb[:, r * NST:(r + 1) * NST], xt_full(r),
                        start=(r == 0),
                        stop=(r == R - 1 and s_prev is None),
                    )
                if s_prev is not None:
                    nc.tensor.matmul(psum_s, a4t_sb, s_prev, start=False,
                                     stop=True)
                s_next = spool.tile([NST, 2 * P], BF16, tag="s")
                nc.vector.tensor_copy(s_next, psum_s)

                y_group(1)
                s_prev = s_next

                # psum -> sbuf (bf16) -> DRAM, halves on separate engines;
                # last window: quarter-granular so copies/stores pipeline
                if w == NW - 1:
                    H = W // 2
                    for g, ceng in ((0, nc.vector.tensor_copy), (1, nc.scalar.copy)):
                        for h in (0, 1):
                            c0 = g * W + h * H
                            ceng(y_sb[:, c0:c0 + H],
                                 psum_y[g][:, h * H:(h + 1) * H])
                            dma_eng[(g + h) % 2].dma_start(
                                y_d[:, w * 2 * W + c0: w * 2 * W + c0 + H],
                                y_sb[:, c0:c0 + H],
                            )
                else:
                    nc.vector.tensor_copy(y_sb[:, 0:W], psum_y[0])
                    dma_eng[w % 2].dma_start(
                        y_d[:, w * 2 * W: w * 2 * W + W], y_sb[:, 0:W]
                    )
                    nc.scalar.copy(y_sb[:, W:2 * W], psum_y[1])
                    dma_eng[1 - w % 2].dma_start(
                        y_d[:, w * 2 * W + W:(w + 1) * 2 * W], y_sb[:, W:2 * W]
                    )
    nc.compile()
    return nc


_NC_CACHE = None
LAST_RESULTS = None  # BassKernelResults of the most recent kernel() call


def _get_nc():
    global _NC_CACHE
    if _NC_CACHE is None:
        _NC_CACHE = _build_nc()
    return _NC_CACHE


def kernel(x: np.ndarray, sos: np.ndarray) -> np.ndarray:
    x = np.asarray(x)
    orig_shape = x.shape
    orig_dtype = x.dtype
    THW, ZA, FTR, A4T = _build_matrices(np.asarray(sos, dtype=np.float64))

    bf = lambda a: np.ascontiguousarray(np.asarray(a, dtype=NPBF16))
    ctab = np.zeros((P, 2 * R * P + R * NST + NST), np.float64)
    ctab[:, 0:R * P] = THW
    ctab[:, R * P:R * P + R * NST] = FTR
    ctab[0:NST, R * P + R * NST:2 * R * P + R * NST] = ZA
    ctab[0:NST, 2 * R * P + R * NST:] = A4T
    ctab = bf(ctab)

    # [core, sig, w, r, p] -> [core, p, w, r, sig]
    xr = x.reshape(NCORES, SPC, NW, R, P).transpose(0, 4, 2, 3, 1)
    xt = bf(xr).reshape(NCORES, P, NW * R * SPC)

    in_maps = [
        {"xt": xt[c], "ctab": ctab}
        for c in range(NCORES)
    ]
    nc = _get_nc()
    res = run_bass_kernel_spmd(nc, in_maps, core_ids=list(range(NCORES)))
    global LAST_RESULTS
    LAST_RESULTS = res
    # y_d [128 p, 8 w, 2 g, 512 c] -> y[core, g*128+p, w*512+c]
    y = np.stack([
        np.asarray(res.results[c]["y"])
        .reshape(P, NW, 2, W)
        .transpose(2, 0, 1, 3)
        .reshape(SPC, T)
        for c in range(NCORES)
    ])
    return y.reshape(orig_shape).astype(orig_dtype, copy=False)


# revision 22
# speedup vs baseline: 1.0879x; 1.0879x over previous
"""Butterworth bandpass (cascaded biquad IIR) Trainium2 kernel.

Problem: y = sosfilt(sos, x) over x[32, 64, 4096] fp32 -- 2048 independent
signals, 4 cascaded DF2T biquads, sequential over T=4096.

Strategy (exact block-parallel reformulation, bf16 data path):
  The cascade is a linear state-space system (A[8,8], B, C, D).  Split T into
  blocks of L=128, grouped in windows of R=4 blocks.  With s = state at the
  window entry, for block r of the window (operators precomputed on host in
  float64 from the 24 sos coefficients):
      y_r = Th @ x_r + sum_{r'<r} (Z A_L^{r-r'-1} F) @ x_{r'} + (Z A_L^r) @ s
      s'  = A_L^R @ s + sum_r (A_L^{R-1-r} F) @ x_r
  All device work is TensorE matmuls over [signal, time] tiles in bf16
  (1 cyc/row at any free size; fp32 PSUM accumulation):
    - the host pre-transposes x into xT block layout [128 time, w, r, sig],
      so the device does no transposes at all and input DMAs are flat
      contiguous 2KB-per-partition lines;
    - one fused rhs table THW[128, 512] = [Th | ZF | ZA_LF | ZA_L^2F] turns
      conv + intra-window cross-block corrections into a single accumulated
      matmul per source block (lhsT = xT_r, N = 512-128r);
    - entry-state corrections for all 4 blocks come from one matmul with
      rhs ZA[8, 512] (lhsT = s);
    - the state update accumulates in a [8, 256] psum; the per-window PE
      order is (g0: ZA+conv) (state) (g1: ZA+conv) so the cross-window
      state copy lands while g1 streams.
  y is written back in a partition-major bf16 layout (flat 1KB DMA lines)
  and un-permuted + upcast on the host.  2048 signals are sharded 256 per
  NeuronCore (two groups of 128 output partitions).
"""

import ml_dtypes
import numpy as np

import concourse.bass as bass
import concourse.tile as tile
from concourse import bacc
from concourse import mybir
from concourse.bass_utils import run_bass_kernel_spmd

FP32 = mybir.dt.float32
BF16 = mybir.dt.bfloat16
NPBF16 = ml_dtypes.bfloat16

P = 128            # partition width == time-block length
T = 4096
NCORES = 8
NSIG = 2048        # 32*64 independent signals
SPC = NSIG // NCORES   # 256 signals per core
NST = 8            # state dim of the 4-biquad cascade
R = 4              # blocks per window
W = P * R          # 512 time steps per window
NW = T // W        # 8 windows


# ----------------------------------------------------------------------------
# host-side: derive block-filter matrices from sos
# ----------------------------------------------------------------------------

def _build_system(sos):
    """Cascade of biquads (DF2T) -> single state space (A, B, C, D), float64."""
    sos = np.asarray(sos, dtype=np.float64)
    A = np.zeros((0, 0))
    B = np.zeros((0,))
    C = np.zeros((0,))
    D = 1.0
    for (b0, b1, b2, _one, a1, a2) in sos:
        As = np.array([[-a1, 1.0], [-a2, 0.0]])
        Bs = np.array([b1 - a1 * b0, b2 - a2 * b0])
        Cs = np.array([1.0, 0.0])
        Ds = b0
        n = A.shape[0]
        Anew = np.zeros((n + 2, n + 2))
        Anew[:n, :n] = A
        Anew[n:, :n] = np.outer(Bs, C)
        Anew[n:, n:] = As
        A = Anew
        B = np.concatenate([B, Bs * D])
        C = np.concatenate([Ds * C, Cs])
        D = Ds * D
    return A, B, C, D


def _balance(A, B, C):
    """Square-root balanced realization: both gramians become diagonal and
    equal, minimizing intermediate-magnitude disparity (important because
    bf16 matmul operands are rounded; unbalanced states reach |s|~650 and
    the rounding noise then dwarfs the O(1) output)."""
    P = np.outer(B, B)
    Ak = A.copy()
    for _ in range(64):
        P = P + Ak @ P @ Ak.T
        Ak = Ak @ Ak
    Q = np.outer(C, C)
    Ak = A.copy()
    for _ in range(64):
        Q = Q + Ak.T @ Q @ Ak
        Ak = Ak @ Ak
    Rc = np.linalg.cholesky(P + 1e-30 * np.eye(len(B)))
    M = Rc.T @ Q @ Rc
    lam, U = np.linalg.eigh(M)
    lam = np.maximum(lam, 1e-30)
    Tm = Rc @ U @ np.diag(lam ** -0.25)
    Ti = np.diag(lam ** 0.25) @ U.T @ np.linalg.inv(Rc)
    return Ti @ A @ Tm, Ti @ B, C @ Tm


def _build_matrices(sos):
    """Window-fused operator tables, float64 -> caller casts to bf16.

    THW[128, 512]: cols [128d:128d+128] = Th (d=0) or (Z A_L^(d-1) F)^T (d>=1)
    ZA [8, 512]:   cols [128r:128r+128] = (Z A_L^r)^T
    FTR[128, 32]:  cols [8r:8r+8]       = ((A_L^(R-1-r)) F)^T
    A4T[8, 8]:     (A_L^R)^T
    """
    A, B, C, D = _build_system(sos)
    A, B, C = _balance(A, B, C)
    ns = A.shape[0]
    assert ns == NST

    h = np.zeros(P)
    h[0] = D
    An = np.eye(ns)
    for k in range(1, P):
        h[k] = C @ An @ B
        An = An @ A
    Th = np.zeros((P, P))
    for m in range(P):
        Th[m, m:] = h[: P - m]

    Z = np.zeros((P, ns))
    CAn = C.copy()
    for n in range(P):
        Z[n] = CAn
        CAn = CAn @ A

    F = np.zeros((ns, P))
    AmB = B.copy()
    for m in range(P - 1, -1, -1):
        F[:, m] = AmB
        AmB = A @ AmB

    AL = np.linalg.matrix_power(A, P)

    THW = np.zeros((P, R * P))
    THW[:, :P] = Th
    for d in range(1, R):
        THW[:, d * P:(d + 1) * P] = (Z @ np.linalg.matrix_power(AL, d - 1) @ F).T
    ZA = np.zeros((ns, R * P))
    for r in range(R):
        ZA[:, r * P:(r + 1) * P] = (Z @ np.linalg.matrix_power(AL, r)).T
    FTR = np.zeros((P, R * NST))
    for r in range(R):
        FTR[:, r * NST:(r + 1) * NST] = (np.linalg.matrix_power(AL, R - 1 - r) @ F).T
    A4T = np.linalg.matrix_power(AL, R).T
    return THW, ZA, FTR, A4T


# ----------------------------------------------------------------------------
# device kernel
# ----------------------------------------------------------------------------

# x chunk split (windows per DMA) interleaved across the two HWDGE engines:
# sync gets w0 alone so the first conv can start ASAP.
# x chunks at 128-time-step block granularity (32 blocks total; block
# b = window b//4, intra-window r = b%4), interleaved across the two HWDGE
# engines.  Window 0 is split across both so its data lands soonest.
XCHUNKS = [  # (engine_idx, b_lo, b_hi)
    (0, 0, 2),     # w0 r0-r1
    (1, 2, 4),     # w0 r2-r3
    (1, 4, 8),     # w1
    (0, 8, 12),    # w2
    (1, 12, 20),   # w3-w4
    (0, 20, 28),   # w5-w6
    (1, 28, 32),   # w7
]


def _build_nc():
    nc = bacc.Bacc("TRN2", target_bir_lowering=False)
    # xt layout: [128 tpos, (8 w, 4 r, 256 s)]  -- element [p, w, r, s]
    xt_d = nc.dram_tensor("xt", [P, NW * R * SPC], BF16, kind="ExternalInput").ap()
    # [THW | FTR | ZA (rows 0:8) | A4T (rows 0:8)] in one table
    CT = R * P + R * NST + R * P + NST
    ctab_d = nc.dram_tensor("ctab", [P, CT], BF16, kind="ExternalInput").ap()
    # y layout: [128 ps, (8 w, 2 g, 512 c)] -- element [p, w, g, c]
    y_d = nc.dram_tensor("y", [P, NW * 2 * W], BF16, kind="ExternalOutput").ap()

    NWARM = 14  # p-state warm-up matmuls bridging the initial DMA wait

    WCOL = R * SPC  # xt columns per window (1024)

    with tile.TileContext(nc) as tc:
        with (
            tc.tile_pool(name="consts", bufs=1) as consts,
            tc.tile_pool(name="ypool", bufs=3) as ypool,
            tc.tile_pool(name="spool", bufs=3) as spool,
            tc.tile_pool(name="py", bufs=2, space="PSUM") as pyp,
            tc.tile_pool(name="ps", bufs=2, space="PSUM") as psp,
            tc.tile_pool(name="pw", bufs=1, space="PSUM") as pwp,
        ):
            dma_eng = (nc.sync, nc.scalar)
            # constant tables first (tiny; they gate the first matmuls),
            # then the x window chunks
            ctab_sb = consts.tile([P, CT], BF16)
            nc.sync.dma_start(ctab_sb, ctab_d)
            thw_sb = ctab_sb[:, 0:R * P]
            ftr_sb = ctab_sb[:, R * P:R * P + R * NST]
            za_sb = ctab_sb[0:NST, R * P + R * NST:2 * R * P + R * NST]
            a4t_sb = ctab_sb[0:NST, 2 * R * P + R * NST:]
            xblk = [[None] * R for _ in range(NW)]
            for eng, b_lo, b_hi in XCHUNKS:
                t = consts.tile([P, (b_hi - b_lo) * SPC], BF16, name=f"xb{b_lo}")
                dma_eng[eng].dma_start(
                    t, xt_d[:, b_lo * SPC:b_hi * SPC]
                )
                for b in range(b_lo, b_hi):
                    xblk[b // R][b % R] = t[:, (b - b_lo) * SPC:(b - b_lo + 1) * SPC]

            # warm-up: keep the PE continuously busy through the DVFS ramp
            # (max clock needs ~3us of gap-free execution) while the first
            # x/ctab DMAs are in flight.  Zeroed operands, result unused.
            warm_sb = consts.tile([P, 3 * P], BF16, name="warm")
            nc.gpsimd.memset(warm_sb, 0)
            psum_warm = pwp.tile([P, 3 * P], FP32, tag="warm")
            for _ in range(NWARM):
                nc.tensor.matmul(
                    psum_warm, warm_sb[:, 0:P], warm_sb, start=True, stop=True,
                )

            s_prev = None  # window 0 enters with zero state

            for w in range(NW):
                blocks = xblk[w]

                def xt_g(r, g):  # [128, 128] lhsT for group g, block r
                    return blocks[r][:, g * P:(g + 1) * P]

                def xt_full(r):  # [128, 256] rhs for the state update
                    return blocks[r]

                psum_y = [
                    pyp.tile([P, W], FP32, tag=f"py{g}", name=f"py{g}")
                    for g in (0, 1)
                ]
                y_sb = ypool.tile([P, 2 * W], BF16, tag="y", name="y_sb")

                # y psum for group g: ZA @ s + conv, every matmul split at
                # column 256 -- N=512 matmuls stream at 1.5 cyc/row on HW
                # while N<=384 run at 1.0, so two N=256 halves are faster.
                def y_group(g):
                    first = s_prev is None
                    if not first:
                        nc.tensor.matmul(
                            psum_y[g], s_prev[:, g * P:(g + 1) * P],
                            za_sb, start=True, stop=False,
                        )
                    for r in range(R):
                        nc.tensor.matmul(
                            psum_y[g][:, r * P:],
                            xt_g(r, g),
                            thw_sb[:, : (R - r) * P],
                            start=(first and r == 0),
                            stop=(r == R - 1),
                        )

                y_group(0)

                # state update (before g1 so the s copy hides under g1);
                # the last window's exit state is never used -- skip it
                if w < NW - 1:
                    psum_s = psp.tile([NST, 2 * P], FP32, tag="ps")
                    if s_prev is not None:
                        nc.tensor.matmul(psum_s, a4t_sb, s_prev,
                                         start=True, stop=False)
                    for r in range(R):
                        nc.tensor.matmul(
                            psum_s, ftr_sb[:, r * NST:(r + 1) * NST], xt_full(r),
                            start=(r == 0 and s_prev is None),
                            stop=(r == R - 1),
                        )
                    s_next = spool.tile([NST, 2 * P], BF16, tag="s")
                    nc.vector.tensor_copy(s_next, psum_s)
                else:
                    s_next = s_prev

                y_group(1)
                s_prev = s_next

                # psum -> sbuf (bf16) -> DRAM, halves on separate engines;
                # last window: quarter-granular so copies/stores pipeline
                if w == NW - 1:
                    H = W // 2
                    for g, ceng in ((0, nc.vector.tensor_copy), (1, nc.scalar.copy)):
                        for h in (0, 1):
                            c0 = g * W + h * H
                            ceng(y_sb[:, c0:c0 + H],
                                 psum_y[g][:, h * H:(h + 1) * H])
                            dma_eng[(g + h) % 2].dma_start(
                                y_d[:, w * 2 * W + c0: w * 2 * W + c0 + H],
                                y_sb[:, c0:c0 + H],
                            )
                else:
                    nc.vector.tensor_copy(y_sb[:, 0:W], psum_y[0])
                    dma_eng[w % 2].dma_start(
                        y_d[:, w * 2 * W: w * 2 * W + W], y_sb[:, 0:W]
                    )
                    nc.scalar.copy(y_sb[:, W:2 * W], psum_y[1])
                    dma_eng[1 - w % 2].dma_start(
                        y_d[:, w * 2 * W + W:(w + 1) * 2 * W], y_sb[:, W:2 * W]
                    )
    nc.compile()
    return nc


_NC_CACHE = None
LAST_RESULTS = None  # BassKernelResults of the most recent kernel() call


def _get_nc():
    global _NC_CACHE
    if _NC_CACHE is None:
        _NC_CACHE = _build_nc()
    return _NC_CACHE


def kernel(x: np.ndarray, sos: np.ndarray) -> np.ndarray:
    x = np.asarray(x)
    orig_shape = x.shape
    orig_dtype = x.dtype
    THW, ZA, FTR, A4T = _build_matrices(np.asarray(sos, dtype=np.float64))

    bf = lambda a: np.ascontiguousarray(np.asarray(a, dtype=NPBF16))
    ctab = np.zeros((P, 2 * R * P + R * NST + NST), np.float64)
    ctab[:, 0:R * P] = THW
    ctab[:, R * P:R * P + R * NST] = FTR
    ctab[0:NST, R * P + R * NST:2 * R * P + R * NST] = ZA
    ctab[0:NST, 2 * R * P + R * NST:] = A4T
    ctab = bf(ctab)

    # [core, sig, w, r, p] -> [core, p, w, r, sig]
    xr = x.reshape(NCORES, SPC, NW, R, P).transpose(0, 4, 2, 3, 1)
    xt = bf(xr).reshape(NCORES, P, NW * R * SPC)

    in_maps = [
        {"xt": xt[c], "ctab": ctab}
        for c in range(NCORES)
    ]
    nc = _get_nc()
    res = run_bass_kernel_spmd(nc, in_maps, core_ids=list(range(NCORES)))
    global LAST_RESULTS
    LAST_RESULTS = res
    # y_d [128 p, 8 w, 2 g, 512 c] -> y[core, g*128+p, w*512+c]
    y = np.stack([
        np.asarray(res.results[c]["y"])
        .reshape(P, NW, 2, W)
        .transpose(2, 0, 1, 3)
        .reshape(SPC, T)
        for c in range(NCORES)
    ])
    return y.reshape(orig_shape).astype(orig_dtype, copy=False)


# revision 24
# speedup vs baseline: 1.0891x; 1.0011x over previous
"""Butterworth bandpass (cascaded biquad IIR) Trainium2 kernel.

Problem: y = sosfilt(sos, x) over x[32, 64, 4096] fp32 -- 2048 independent
signals, 4 cascaded DF2T biquads, sequential over T=4096.

Strategy (exact block-parallel reformulation, bf16 data path):
  The cascade is a linear state-space system (A[8,8], B, C, D).  Split T into
  blocks of L=128, grouped in windows of R=4 blocks.  With s = state at the
  window entry, for block r of the window (operators precomputed on host in
  float64 from the 24 sos coefficients):
      y_r = Th @ x_r + sum_{r'<r} (Z A_L^{r-r'-1} F) @ x_{r'} + (Z A_L^r) @ s
      s'  = A_L^R @ s + sum_r (A_L^{R-1-r} F) @ x_r
  All device work is TensorE matmuls over [signal, time] tiles in bf16
  (1 cyc/row at any free size; fp32 PSUM accumulation):
    - the host pre-transposes x into xT block layout [128 time, w, r, sig],
      so the device does no transposes at all and input DMAs are flat
      contiguous 2KB-per-partition lines;
    - one fused rhs table THW[128, 512] = [Th | ZF | ZA_LF | ZA_L^2F] turns
      conv + intra-window cross-block corrections into a single accumulated
      matmul per source block (lhsT = xT_r, N = 512-128r);
    - entry-state corrections for all 4 blocks come from one matmul with
      rhs ZA[8, 512] (lhsT = s);
    - the state update accumulates in a [8, 256] psum; the per-window PE
      order is (g0: ZA+conv) (state) (g1: ZA+conv) so the cross-window
      state copy lands while g1 streams.
  y is written back in a partition-major bf16 layout (flat 1KB DMA lines)
  and un-permuted + upcast on the host.  2048 signals are sharded 256 per
  NeuronCore (two groups of 128 output partitions).
"""

import ml_dtypes
import numpy as np

import concourse.bass as bass
import concourse.tile as tile
from concourse import bacc
from concourse import mybir
from concourse.bass_utils import run_bass_kernel_spmd

FP32 = mybir.dt.float32
BF16 = mybir.dt.bfloat16
NPBF16 = ml_dtypes.bfloat16

P = 128            # partition width == time-block length
T = 4096
NCORES = 8
NSIG = 2048        # 32*64 independent signals
SPC = NSIG // NCORES   # 256 signals per core
NST = 8            # state dim of the 4-biquad cascade
R = 4              # blocks per window
W = P * R          # 512 time steps per window
NW = T // W        # 8 windows


# ----------------------------------------------------------------------------
# host-side: derive block-filter matrices from sos
# ----------------------------------------------------------------------------

def _build_system(sos):
    """Cascade of biquads (DF2T) -> single state space (A, B, C, D), float64."""
    sos = np.asarray(sos, dtype=np.float64)
    A = np.zeros((0, 0))
    B = np.zeros((0,))
    C = np.zeros((0,))
    D = 1.0
    for (b0, b1, b2, _one, a1, a2) in sos:
        As = np.array([[-a1, 1.0], [-a2, 0.0]])
        Bs = np.array([b1 - a1 * b0, b2 - a2 * b0])
        Cs = np.array([1.0, 0.0])
        Ds = b0
        n = A.shape[0]
        Anew = np.zeros((n + 2, n + 2))
        Anew[:n, :n] = A
        Anew[n:, :n] = np.outer(Bs, C)
        Anew[n:, n:] = As
        A = Anew
        B = np.concatenate([B, Bs * D])
        C = np.concatenate([Ds * C, Cs])
        D = Ds * D
    return A, B, C, D


def _balance(A, B, C):
    """Square-root balanced realization: both gramians become diagonal and
    equal, minimizing intermediate-magnitude disparity (important because
    bf16 matmul operands are rounded; unbalanced states reach |s|~650 and
    the rounding noise then dwarfs the O(1) output)."""
    P = np.outer(B, B)
    Ak = A.copy()
    for _ in range(64):
        P = P + Ak @ P @ Ak.T
        Ak = Ak @ Ak
    Q = np.outer(C, C)
    Ak = A.copy()
    for _ in range(64):
        Q = Q + Ak.T @ Q @ Ak
        Ak = Ak @ Ak
    Rc = np.linalg.cholesky(P + 1e-30 * np.eye(len(B)))
    M = Rc.T @ Q @ Rc
    lam, U = np.linalg.eigh(M)
    lam = np.maximum(lam, 1e-30)
    Tm = Rc @ U @ np.diag(lam ** -0.25)
    Ti = np.diag(lam ** 0.25) @ U.T @ np.linalg.inv(Rc)
    return Ti @ A @ Tm, Ti @ B, C @ Tm


def _build_matrices(sos):
    """Window-fused operator tables, float64 -> caller casts to bf16.

    THW[128, 512]: cols [128d:128d+128] = Th (d=0) or (Z A_L^(d-1) F)^T (d>=1)
    ZA [8, 512]:   cols [128r:128r+128] = (Z A_L^r)^T
    FTR[128, 32]:  cols [8r:8r+8]       = ((A_L^(R-1-r)) F)^T
    A4T[8, 8]:     (A_L^R)^T
    """
    A, B, C, D = _build_system(sos)
    A, B, C = _balance(A, B, C)
    ns = A.shape[0]
    assert ns == NST

    h = np.zeros(P)
    h[0] = D
    An = np.eye(ns)
    for k in range(1, P):
        h[k] = C @ An @ B
        An = An @ A
    Th = np.zeros((P, P))
    for m in range(P):
        Th[m, m:] = h[: P - m]

    Z = np.zeros((P, ns))
    CAn = C.copy()
    for n in range(P):
        Z[n] = CAn
        CAn = CAn @ A

    F = np.zeros((ns, P))
    AmB = B.copy()
    for m in range(P - 1, -1, -1):
        F[:, m] = AmB
        AmB = A @ AmB

    AL = np.linalg.matrix_power(A, P)

    THW = np.zeros((P, R * P))
    THW[:, :P] = Th
    for d in range(1, R):
        THW[:, d * P:(d + 1) * P] = (Z @ np.linalg.matrix_power(AL, d - 1) @ F).T
    ZA = np.zeros((ns, R * P))
    for r in range(R):
        ZA[:, r * P:(r + 1) * P] = (Z @ np.linalg.matrix_power(AL, r)).T
    FTR = np.zeros((P, R * NST))
    for r in range(R):
        FTR[:, r * NST:(r + 1) * NST] = (np.linalg.matrix_power(AL, R - 1 - r) @ F).T
    A4T = np.linalg.matrix_power(AL, R).T
    return THW, ZA, FTR, A4T


# ----------------------------------------------------------------------------
# device kernel
# ----------------------------------------------------------------------------

# x chunk split (windows per DMA) interleaved across the two HWDGE engines:
# sync gets w0 alone so the first conv can start ASAP.
# x chunks at 128-time-step block granularity (32 blocks total; block
# b = window b//4, intra-window r = b%4), interleaved across the two HWDGE
# engines.  Window 0 is split across both so its data lands soonest.
XCHUNKS = [  # (engine_idx, b_lo, b_hi)
    (0, 0, 2),     # w0 r0-r1
    (1, 2, 4),     # w0 r2-r3
    (1, 4, 8),     # w1
    (0, 8, 12),    # w2
    (1, 12, 20),   # w3-w4
    (0, 20, 28),   # w5-w6
    (1, 28, 32),   # w7
]


def _build_nc():
    nc = bacc.Bacc("TRN2", target_bir_lowering=False)
    # xt layout: [128 tpos, (8 w, 4 r, 256 s)]  -- element [p, w, r, s]
    xt_d = nc.dram_tensor("xt", [P, NW * R * SPC], BF16, kind="ExternalInput").ap()
    # [THW | FTR | ZA (rows 0:8) | A4T (rows 0:8)] in one table
    CT = R * P + R * NST + R * P + NST
    ctab_d = nc.dram_tensor("ctab", [P, CT], BF16, kind="ExternalInput").ap()
    # y layout: [128 ps, (8 w, 2 g, 512 c)] -- element [p, w, g, c]
    y_d = nc.dram_tensor("y", [P, NW * 2 * W], BF16, kind="ExternalOutput").ap()

    NWARM = 11  # p-state warm-up matmuls bridging the initial DMA wait

    WCOL = R * SPC  # xt columns per window (1024)

    with tile.TileContext(nc) as tc:
        with (
            tc.tile_pool(name="consts", bufs=1) as consts,
            tc.tile_pool(name="ypool", bufs=3) as ypool,
            tc.tile_pool(name="spool", bufs=3) as spool,
            tc.tile_pool(name="py", bufs=2, space="PSUM") as pyp,
            tc.tile_pool(name="ps", bufs=2, space="PSUM") as psp,
            tc.tile_pool(name="pw", bufs=1, space="PSUM") as pwp,
        ):
            dma_eng = (nc.sync, nc.scalar)
            # constant tables first (tiny; they gate the first matmuls),
            # then the x window chunks
            ctab_sb = consts.tile([P, CT], BF16)
            nc.sync.dma_start(ctab_sb, ctab_d)
            thw_sb = ctab_sb[:, 0:R * P]
            ftr_sb = ctab_sb[:, R * P:R * P + R * NST]
            za_sb = ctab_sb[0:NST, R * P + R * NST:2 * R * P + R * NST]
            a4t_sb = ctab_sb[0:NST, 2 * R * P + R * NST:]
            xblk = [[None] * R for _ in range(NW)]
            for eng, b_lo, b_hi in XCHUNKS:
                t = consts.tile([P, (b_hi - b_lo) * SPC], BF16, name=f"xb{b_lo}")
                dma_eng[eng].dma_start(
                    t, xt_d[:, b_lo * SPC:b_hi * SPC]
                )
                for b in range(b_lo, b_hi):
                    xblk[b // R][b % R] = t[:, (b - b_lo) * SPC:(b - b_lo + 1) * SPC]

            # warm-up: keep the PE continuously busy through the DVFS ramp
            # (max clock needs ~3us of gap-free execution) while the first
            # x/ctab DMAs are in flight.  Zeroed operands, result unused.
            warm_sb = consts.tile([P, 3 * P], BF16, name="warm")
            nc.gpsimd.memset(warm_sb, 0)
            psum_warm = pwp.tile([P, 3 * P], FP32, tag="warm")
            for _ in range(NWARM):
                nc.tensor.matmul(
                    psum_warm, warm_sb[:, 0:P], warm_sb, start=True, stop=True,
                )

            s_prev = None  # window 0 enters with zero state

            for w in range(NW):
                blocks = xblk[w]

                def xt_g(r, g):  # [128, 128] lhsT for group g, block r
                    return blocks[r][:, g * P:(g + 1) * P]

                def xt_full(r):  # [128, 256] rhs for the state update
                    return blocks[r]

                psum_y = [
                    pyp.tile([P, W], FP32, tag=f"py{g}", name=f"py{g}")
                    for g in (0, 1)
                ]
                y_sb = ypool.tile([P, 2 * W], BF16, tag="y", name="y_sb")

                # y psum for group g: ZA @ s + conv, every matmul split at
                # column 256 -- N=512 matmuls stream at 1.5 cyc/row on HW
                # while N<=384 run at 1.0, so two N=256 halves are faster.
                def y_group(g):
                    first = s_prev is None
                    if not first:
                        nc.tensor.matmul(
                            psum_y[g], s_prev[:, g * P:(g + 1) * P],
                            za_sb, start=True, stop=False,
                        )
                    for r in range(R):
                        nc.tensor.matmul(
                            psum_y[g][:, r * P:],
                            xt_g(r, g),
                            thw_sb[:, : (R - r) * P],
                            start=(first and r == 0),
                            stop=(r == R - 1),
                        )

                y_group(0)

                # state update (before g1 so the s copy hides under g1);
                # the last window's exit state is never used -- skip it
                if w < NW - 1:
                    psum_s = psp.tile([NST, 2 * P], FP32, tag="ps")
                    if s_prev is not None:
                        nc.tensor.matmul(psum_s, a4t_sb, s_prev,
                                         start=True, stop=False)
                    for r in range(R):
                        nc.tensor.matmul(
                            psum_s, ftr_sb[:, r * NST:(r + 1) * NST], xt_full(r),
                            start=(r == 0 and s_prev is None),
                            stop=(r == R - 1),
                        )
                    s_next = spool.tile([NST, 2 * P], BF16, tag="s")
                    nc.vector.tensor_copy(s_next, psum_s)
                else:
                    s_next = s_prev

                y_group(1)
                s_prev = s_next

                # psum -> sbuf (bf16) -> DRAM, halves on separate engines;
                # last window: quarter-granular so copies/stores pipeline
                if w == NW - 1:
                    H = W // 2
                    for g, ceng in ((0, nc.vector.tensor_copy), (1, nc.scalar.copy)):
                        for h in (0, 1):
                            c0 = g * W + h * H
                            ceng(y_sb[:, c0:c0 + H],
                                 psum_y[g][:, h * H:(h + 1) * H])
                            dma_eng[(g + h) % 2].dma_start(
                                y_d[:, w * 2 * W + c0: w * 2 * W + c0 + H],
                                y_sb[:, c0:c0 + H],
                            )
                else:
                    # mid-kernel stores go out via SWDGE (gpsimd) so the two
                    # HWDGE sequencers never queue descriptor-gen work behind
                    # the latency-critical last-window copies/stores
                    nc.vector.tensor_copy(y_sb[:, 0:W], psum_y[0])
                    nc.gpsimd.dma_start(
                        y_d[:, w * 2 * W: w * 2 * W + W], y_sb[:, 0:W]
                    )
                    nc.scalar.copy(y_sb[:, W:2 * W], psum_y[1])
                    nc.gpsimd.dma_start(
                        y_d[:, w * 2 * W + W:(w + 1) * 2 * W], y_sb[:, W:2 * W]
                    )
    nc.compile()
    return nc


_NC_CACHE = None
LAST_RESULTS = None  # BassKernelResults of the most recent kernel() call


def _get_nc():
    global _NC_CACHE
    if _NC_CACHE is None:
        _NC_CACHE = _build_nc()
    return _NC_CACHE


def kernel(x: np.ndarray, sos: np.ndarray) -> np.ndarray:
    x = np.asarray(x)
    orig_shape = x.shape
    orig_dtype = x.dtype
    THW, ZA, FTR, A4T = _build_matrices(np.asarray(sos, dtype=np.float64))

    bf = lambda a: np.ascontiguousarray(np.asarray(a, dtype=NPBF16))
    ctab = np.zeros((P, 2 * R * P + R * NST + NST), np.float64)
    ctab[:, 0:R * P] = THW
    ctab[:, R * P:R * P + R * NST] = FTR
    ctab[0:NST, R * P + R * NST:2 * R * P + R * NST] = ZA
    ctab[0:NST, 2 * R * P + R * NST:] = A4T
    ctab = bf(ctab)

    # [core, sig, w, r, p] -> [core, p, w, r, sig]
    xr = x.reshape(NCORES, SPC, NW, R, P).transpose(0, 4, 2, 3, 1)
    xt = bf(xr).reshape(NCORES, P, NW * R * SPC)

    in_maps = [
        {"xt": xt[c], "ctab": ctab}
        for c in range(NCORES)
    ]
    nc = _get_nc()
    res = run_bass_kernel_spmd(nc, in_maps, core_ids=list(range(NCORES)))
    global LAST_RESULTS
    LAST_RESULTS = res
    # y_d [128 p, 8 w, 2 g, 512 c] -> y[core, g*128+p, w*512+c]
    y = np.stack([
        np.asarray(res.results[c]["y"])
        .reshape(P, NW, 2, W)
        .transpose(2, 0, 1, 3)
        .reshape(SPC, T)
        for c in range(NCORES)
    ])
    return y.reshape(orig_shape).astype(orig_dtype, copy=False)


# revision 26
# speedup vs baseline: 1.1067x; 1.0161x over previous
"""Butterworth bandpass (cascaded biquad IIR) Trainium2 kernel.

Problem: y = sosfilt(sos, x) over x[32, 64, 4096] fp32 -- 2048 independent
signals, 4 cascaded DF2T biquads, sequential over T=4096.

Strategy (exact block-parallel reformulation, bf16 data path):
  The cascade is a linear state-space system (A[8,8], B, C, D).  Split T into
  blocks of L=128, grouped in windows of R=4 blocks.  With s = state at the
  window entry, for block r of the window (operators precomputed on host in
  float64 from the 24 sos coefficients):
      y_r = Th @ x_r + sum_{r'<r} (Z A_L^{r-r'-1} F) @ x_{r'} + (Z A_L^r) @ s
      s'  = A_L^R @ s + sum_r (A_L^{R-1-r} F) @ x_r
  All device work is TensorE matmuls over [signal, time] tiles in bf16
  (1 cyc/row at any free size; fp32 PSUM accumulation):
    - the host pre-transposes x into xT block layout [128 time, w, r, sig],
      so the device does no transposes at all and input DMAs are flat
      contiguous 2KB-per-partition lines;
    - one fused rhs table THW[128, 512] = [Th | ZF | ZA_LF | ZA_L^2F] turns
      conv + intra-window cross-block corrections into a single accumulated
      matmul per source block (lhsT = xT_r, N = 512-128r);
    - entry-state corrections for all 4 blocks come from one matmul with
      rhs ZA[8, 512] (lhsT = s);
    - the state update accumulates in a [8, 256] psum; the per-window PE
      order is (g0: ZA+conv) (state) (g1: ZA+conv) so the cross-window
      state copy lands while g1 streams.
  y is written back in a partition-major bf16 layout (flat 1KB DMA lines)
  and un-permuted + upcast on the host.  2048 signals are sharded 256 per
  NeuronCore (two groups of 128 output partitions).
"""

import ml_dtypes
import numpy as np

import concourse.bass as bass
import concourse.tile as tile
from concourse import bacc
from concourse import mybir
from concourse.bass_utils import run_bass_kernel_spmd

FP32 = mybir.dt.float32
BF16 = mybir.dt.bfloat16
NPBF16 = ml_dtypes.bfloat16

P = 128            # partition width == time-block length
T = 4096
NCORES = 8
NSIG = 2048        # 32*64 independent signals
SPC = NSIG // NCORES   # 256 signals per core
NST = 8            # state dim of the 4-biquad cascade
R = 4              # blocks per window
W = P * R          # 512 time steps per window
NW = T // W        # 8 windows


# ----------------------------------------------------------------------------
# host-side: derive block-filter matrices from sos
# ----------------------------------------------------------------------------

def _build_system(sos):
    """Cascade of biquads (DF2T) -> single state space (A, B, C, D), float64."""
    sos = np.asarray(sos, dtype=np.float64)
    A = np.zeros((0, 0))
    B = np.zeros((0,))
    C = np.zeros((0,))
    D = 1.0
    for (b0, b1, b2, _one, a1, a2) in sos:
        As = np.array([[-a1, 1.0], [-a2, 0.0]])
        Bs = np.array([b1 - a1 * b0, b2 - a2 * b0])
        Cs = np.array([1.0, 0.0])
        Ds = b0
        n = A.shape[0]
        Anew = np.zeros((n + 2, n + 2))
        Anew[:n, :n] = A
        Anew[n:, :n] = np.outer(Bs, C)
        Anew[n:, n:] = As
        A = Anew
        B = np.concatenate([B, Bs * D])
        C = np.concatenate([Ds * C, Cs])
        D = Ds * D
    return A, B, C, D


def _balance(A, B, C):
    """Square-root balanced realization: both gramians become diagonal and
    equal, minimizing intermediate-magnitude disparity (important because
    bf16 matmul operands are rounded; unbalanced states reach |s|~650 and
    the rounding noise then dwarfs the O(1) output)."""
    P = np.outer(B, B)
    Ak = A.copy()
    for _ in range(64):
        P = P + Ak @ P @ Ak.T
        Ak = Ak @ Ak
    Q = np.outer(C, C)
    Ak = A.copy()
    for _ in range(64):
        Q = Q + Ak.T @ Q @ Ak
        Ak = Ak @ Ak
    Rc = np.linalg.cholesky(P + 1e-30 * np.eye(len(B)))
    M = Rc.T @ Q @ Rc
    lam, U = np.linalg.eigh(M)
    lam = np.maximum(lam, 1e-30)
    Tm = Rc @ U @ np.diag(lam ** -0.25)
    Ti = np.diag(lam ** 0.25) @ U.T @ np.linalg.inv(Rc)
    return Ti @ A @ Tm, Ti @ B, C @ Tm


def _build_matrices(sos):
    """Window-fused operator tables, float64 -> caller casts to bf16.

    THW[128, 512]: cols [128d:128d+128] = Th (d=0) or (Z A_L^(d-1) F)^T (d>=1)
    ZA [8, 512]:   cols [128r:128r+128] = (Z A_L^r)^T
    FTR[128, 32]:  cols [8r:8r+8]       = ((A_L^(R-1-r)) F)^T
    A4T[8, 8]:     (A_L^R)^T
    """
    A, B, C, D = _build_system(sos)
    A, B, C = _balance(A, B, C)
    ns = A.shape[0]
    assert ns == NST

    h = np.zeros(P)
    h[0] = D
    An = np.eye(ns)
    for k in range(1, P):
        h[k] = C @ An @ B
        An = An @ A
    Th = np.zeros((P, P))
    for m in range(P):
        Th[m, m:] = h[: P - m]

    Z = np.zeros((P, ns))
    CAn = C.copy()
    for n in range(P):
        Z[n] = CAn
        CAn = CAn @ A

    F = np.zeros((ns, P))
    AmB = B.copy()
    for m in range(P - 1, -1, -1):
        F[:, m] = AmB
        AmB = A @ AmB

    AL = np.linalg.matrix_power(A, P)

    THW = np.zeros((P, R * P))
    THW[:, :P] = Th
    for d in range(1, R):
        THW[:, d * P:(d + 1) * P] = (Z @ np.linalg.matrix_power(AL, d - 1) @ F).T
    ZA = np.zeros((ns, R * P))
    for r in range(R):
        ZA[:, r * P:(r + 1) * P] = (Z @ np.linalg.matrix_power(AL, r)).T
    FTR = np.zeros((P, R * NST))
    for r in range(R):
        FTR[:, r * NST:(r + 1) * NST] = (np.linalg.matrix_power(AL, R - 1 - r) @ F).T
    A4T = np.linalg.matrix_power(AL, R).T
    return THW, ZA, FTR, A4T


# ----------------------------------------------------------------------------
# device kernel
# ----------------------------------------------------------------------------

# x chunk split (windows per DMA) interleaved across the two HWDGE engines:
# sync gets w0 alone so the first conv can start ASAP.
# x chunks at 128-time-step block granularity (32 blocks total; block
# b = window b//4, intra-window r = b%4), interleaved across the two HWDGE
# engines.  Window 0 is split across both so its data lands soonest.
XCHUNKS = [  # (engine_idx, b_lo, b_hi)
    (0, 0, 2),     # w0 r0-r1
    (1, 2, 4),     # w0 r2-r3
    (1, 4, 8),     # w1
    (0, 8, 12),    # w2
    (1, 12, 20),   # w3-w4
    (0, 20, 28),   # w5-w6
    (1, 28, 32),   # w7
]


def _build_nc():
    nc = bacc.Bacc("TRN2", target_bir_lowering=False)
    # xt layout: [128 tpos, (8 w, 4 r, 256 s)]  -- element [p, w, r, s]
    xt_d = nc.dram_tensor("xt", [P, NW * R * SPC], BF16, kind="ExternalInput").ap()
    # [THW | FTR | ZA (rows 0:8) | A4T (rows 0:8)] in one table
    CT = R * P + R * NST + R * P + NST
    ctab_d = nc.dram_tensor("ctab", [P, CT], BF16, kind="ExternalInput").ap()
    # y layout: [128 ps, (8 w, 2 g, 512 c)] -- element [p, w, g, c]
    y_d = nc.dram_tensor("y", [P, NW * 2 * W], BF16, kind="ExternalOutput").ap()

    NWARM = 13  # p-state warm-up matmuls bridging the initial DMA wait

    WCOL = R * SPC  # xt columns per window (1024)

    with tile.TileContext(nc) as tc:
        with (
            tc.tile_pool(name="consts", bufs=1) as consts,
            tc.tile_pool(name="ypool", bufs=3) as ypool,
            tc.tile_pool(name="spool", bufs=3) as spool,
            tc.tile_pool(name="py", bufs=2, space="PSUM") as pyp,
            tc.tile_pool(name="ps", bufs=2, space="PSUM") as psp,
            tc.tile_pool(name="pw", bufs=1, space="PSUM") as pwp,
        ):
            dma_eng = (nc.sync, nc.scalar)
            # constant tables first (tiny; they gate the first matmuls),
            # then the x window chunks
            ctab_sb = consts.tile([P, CT], BF16)
            nc.sync.dma_start(ctab_sb, ctab_d)
            thw_sb = ctab_sb[:, 0:R * P]
            ftr_sb = ctab_sb[:, R * P:R * P + R * NST]
            za_sb = ctab_sb[0:NST, R * P + R * NST:2 * R * P + R * NST]
            a4t_sb = ctab_sb[0:NST, 2 * R * P + R * NST:]
            xblk = [[None] * R for _ in range(NW)]
            for eng, b_lo, b_hi in XCHUNKS:
                t = consts.tile([P, (b_hi - b_lo) * SPC], BF16, name=f"xb{b_lo}")
                dma_eng[eng].dma_start(
                    t, xt_d[:, b_lo * SPC:b_hi * SPC]
                )
                for b in range(b_lo, b_hi):
                    xblk[b // R][b % R] = t[:, (b - b_lo) * SPC:(b - b_lo + 1) * SPC]

            # warm-up: keep the PE continuously busy through the DVFS ramp
            # (max clock needs ~3us of gap-free execution) while the first
            # x/ctab DMAs are in flight.  Zeroed operands, result unused.
            warm_sb = consts.tile([P, 3 * P], BF16, name="warm")
            nc.gpsimd.memset(warm_sb, 0)
            psum_warm = pwp.tile([P, 3 * P], FP32, tag="warm")
            for _ in range(NWARM):
                nc.tensor.matmul(
                    psum_warm, warm_sb[:, 0:P], warm_sb, start=True, stop=True,
                )

            s_prev = None  # window 0 enters with zero state

            for w in range(NW):
                blocks = xblk[w]

                def xt_g(r, g):  # [128, 128] lhsT for group g, block r
                    return blocks[r][:, g * P:(g + 1) * P]

                def xt_full(r):  # [128, 256] rhs for the state update
                    return blocks[r]

                psum_y = [
                    pyp.tile([P, W], FP32, tag=f"py{g}", name=f"py{g}")
                    for g in (0, 1)
                ]
                y_sb = ypool.tile([P, 2 * W], BF16, tag="y", name="y_sb")

                # y psum for group g: ZA @ s + conv, every matmul split at
                # column 256 -- N=512 matmuls stream at 1.5 cyc/row on HW
                # while N<=384 run at 1.0, so two N=256 halves are faster.
                def y_group(g):
                    first = s_prev is None
                    if not first:
                        nc.tensor.matmul(
                            psum_y[g], s_prev[:, g * P:(g + 1) * P],
                            za_sb, start=True, stop=False,
                        )
                    for r in range(R):
                        nc.tensor.matmul(
                            psum_y[g][:, r * P:],
                            xt_g(r, g),
                            thw_sb[:, : (R - r) * P],
                            start=(first and r == 0),
                            stop=(r == R - 1),
                        )

                y_group(0)

                # state update (before g1 so the s copy hides under g1);
                # the last window's exit state is never used -- skip it
                if w < NW - 1:
                    psum_s = psp.tile([NST, 2 * P], FP32, tag="ps")
                    if s_prev is not None:
                        nc.tensor.matmul(psum_s, a4t_sb, s_prev,
                                         start=True, stop=False)
                    for r in range(R):
                        nc.tensor.matmul(
                            psum_s, ftr_sb[:, r * NST:(r + 1) * NST], xt_full(r),
                            start=(r == 0 and s_prev is None),
                            stop=(r == R - 1),
                        )
                    s_next = spool.tile([NST, 2 * P], BF16, tag="s")
                    nc.vector.tensor_copy(s_next, psum_s)
                else:
                    s_next = s_prev

                y_group(1)
                s_prev = s_next

                # psum -> sbuf (bf16) -> DRAM, halves on separate engines;
                # last window: each psum is half-copied by BOTH copy engines
                # and stored immediately, so the final store chain is short
                if w == NW - 1:
                    H = W // 2
                    for g in (0, 1):
                        for h, ceng in ((0, nc.vector.tensor_copy),
                                        (1, nc.scalar.copy)):
                            c0 = g * W + h * H
                            ceng(y_sb[:, c0:c0 + H],
                                 psum_y[g][:, h * H:(h + 1) * H])
                            dma_eng[h].dma_start(
                                y_d[:, w * 2 * W + c0: w * 2 * W + c0 + H],
                                y_sb[:, c0:c0 + H],
                            )
                else:
                    # mid-kernel stores go out via SWDGE (gpsimd) so the two
                    # HWDGE sequencers never queue descriptor-gen work behind
                    # the latency-critical last-window copies/stores
                    nc.vector.tensor_copy(y_sb[:, 0:W], psum_y[0])
                    nc.gpsimd.dma_start(
                        y_d[:, w * 2 * W: w * 2 * W + W], y_sb[:, 0:W]
                    )
                    nc.scalar.copy(y_sb[:, W:2 * W], psum_y[1])
                    nc.gpsimd.dma_start(
                        y_d[:, w * 2 * W + W:(w + 1) * 2 * W], y_sb[:, W:2 * W]
                    )
    nc.compile()
    return nc


_NC_CACHE = None
LAST_RESULTS = None  # BassKernelResults of the most recent kernel() call


def _get_nc():
    global _NC_CACHE
    if _NC_CACHE is None:
        _NC_CACHE = _build_nc()
    return _NC_CACHE


def kernel(x: np.ndarray, sos: np.ndarray) -> np.ndarray:
    x = np.asarray(x)
    orig_shape = x.shape
    orig_dtype = x.dtype
    THW, ZA, FTR, A4T = _build_matrices(np.asarray(sos, dtype=np.float64))

    bf = lambda a: np.ascontiguousarray(np.asarray(a, dtype=NPBF16))
    ctab = np.zeros((P, 2 * R * P + R * NST + NST), np.float64)
    ctab[:, 0:R * P] = THW
    ctab[:, R * P:R * P + R * NST] = FTR
    ctab[0:NST, R * P + R * NST:2 * R * P + R * NST] = ZA
    ctab[0:NST, 2 * R * P + R * NST:] = A4T
    ctab = bf(ctab)

    # [core, sig, w, r, p] -> [core, p, w, r, sig]
    xr = x.reshape(NCORES, SPC, NW, R, P).transpose(0, 4, 2, 3, 1)
    xt = bf(xr).reshape(NCORES, P, NW * R * SPC)

    in_maps = [
        {"xt": xt[c], "ctab": ctab}
        for c in range(NCORES)
    ]
    nc = _get_nc()
    res = run_bass_kernel_spmd(nc, in_maps, core_ids=list(range(NCORES)))
    global LAST_RESULTS
    LAST_RESULTS = res
    # y_d [128 p, 8 w, 2 g, 512 c] -> y[core, g*128+p, w*512+c]
    y = np.stack([
        np.asarray(res.results[c]["y"])
        .reshape(P, NW, 2, W)
        .transpose(2, 0, 1, 3)
        .reshape(SPC, T)
        for c in range(NCORES)
    ])
    return y.reshape(orig_shape).astype(orig_dtype, copy=False)
